# revision 16
# baseline (speedup 1.0000x reference)
"""DiffPoolEncoder Trainium2 kernel (v4).

Sharding: data parallel by graph. 8 cores x 4 graphs (512 nodes each).
Datapath: fp16 for the h-stack (feeds the output readout directly, keeps
tf32's 10 mantissa bits; fp32 accumulation in PSUM). The entire assignment
stack (a1/a2 linears + their aggregations + a3 + logits), whose only output
is a 64-way softmax, runs in fp8e4m3 with DoubleRow perf mode (two
contraction rows per PE pass). Dense per-graph A^T tiles are host-prescaled
(cnt/deg, exact in both fp16 and fp8 since deg=16) and DMA'd directly.
PSUM evacuations are split across Act/DVE; fp16->fp8 shadow copies run on
the otherwise-idle GpSimd engine.
"""

import sys

for _p in ("/opt/trn_rl_repo",):
    if _p not in sys.path:
        sys.path.append(_p)

import numpy as np
from contextlib import ExitStack

import concourse.bass as bass
import concourse.mybir as mybir
import concourse.tile as tile
from concourse import bacc
from concourse.bass_utils import run_bass_kernel_spmd

F32 = mybir.dt.float32
F16 = mybir.dt.float16
F8 = mybir.dt.float8e4
DR = mybir.MatmulPerfMode.DoubleRow
AF = mybir.ActivationFunctionType
ALU = mybir.AluOpType
AX = mybir.AxisListType

NCORES = 8
B = 32
NPG = 512
G = 4            # graphs per core
T = 16           # node tiles per core (4 per graph)
NLOC = 2048      # nodes per core
K = 64           # clusters per graph
IN = 128
HID = 256

# bcol column layout (each 128-chunk of a bias vector is one column)
BC_B1, BC_B2, BC_B3 = 0, 2, 4
BC_AB1, BC_AB2 = 6, 8
BC_AB3 = 10          # 16 cols
BC_QB1, BC_QB2, BC_QB3 = 26, 28, 30
BC_MB1, BC_MB2 = 32, 34
BC_N = 35

# rows2 [65, 1024] f16: rows at matmul base partitions {0, 32, 64};
# ones[0:512] replicated at each used partition (matmul needs equal bases).
R_QB1 = (0, 512)
R_QB2, R_QB3 = (64, 512), (64, 768)
R_PB = (32, 512)     # 256 (per-core pW bias slice)
ROWS_W = 1024


def build_module(ab3_zero):
    nc = bacc.Bacc("TRN2", target_bir_lowering=False)

    # ---------------- DRAM I/O ----------------
    featT_d = nc.dram_tensor("featT", [128, NLOC], F16, kind="ExternalInput")
    featT8_d = nc.dram_tensor("featT8", [128, NLOC], F8, kind="ExternalInput")
    featnm_d = nc.dram_tensor("feat_nm", [128, T * IN], F16, kind="ExternalInput")
    at_d = nc.dram_tensor("at_dense", [128, T * NPG], F16, kind="ExternalInput")
    at8_d = nc.dram_tensor("at8", [128, T * NPG], F8, kind="ExternalInput")
    degc_d = nc.dram_tensor("degc", [128, T], F32, kind="ExternalInput")
    bcol_d = nc.dram_tensor("bcol", [128, BC_N], F32, kind="ExternalInput")
    rows_d = nc.dram_tensor("rows2", [65, ROWS_W], F16, kind="ExternalInput")
    identr_d = nc.dram_tensor("identr", [128, 128], F16, kind="ExternalInput")
    w_d = {}
    for name, fi, fo, dt in [
        ("W1", 256, 256, F16), ("W2", 512, 256, F16), ("W3", 512, 256, F16),
        ("aW1", 256, 256, F8), ("aW2", 512, 256, F8), ("aW3", 512, 2048, F8),
        ("pWa", 512, 256, F8), ("pW3", 2048, 256, F8),
        ("qW1", 1536, 256, F16), ("qW2", 512, 256, F16),
        ("qW3", 512, 256, F16), ("mW1", 1536, 256, F16), ("mW2", 256, 10, F16),
    ]:
        w_d[name] = nc.dram_tensor(name, [fi, fo], dt, kind="ExternalInput")
    yp_d = nc.dram_tensor("yp", [10, G], F32, kind="ExternalOutput")

    with tile.TileContext(nc) as tc, ExitStack() as ex, \
            nc.allow_low_precision(reason="fp16/fp8 datapath; accumulation stays fp32 in PSUM"):
        persist = ex.enter_context(tc.tile_pool(name="persist", bufs=1))
        # PSUM: 8 banks = psP 3x1 + a3 pair pool 2x2 + logits 1.
        ps_p = ex.enter_context(tc.tile_pool(name="psP", bufs=3, space="PSUM"))
        a3_p = ex.enter_context(tc.tile_pool(name="psA3", bufs=2, space="PSUM"))
        lg_p = ex.enter_context(tc.tile_pool(name="psL", bufs=1, space="PSUM"))

        uid = [0]

        def _nm(pfx):
            uid[0] += 1
            return f"{pfx}{uid[0]}"

        def ps_big(dt=F32):
            return ps_p.tile([128, 512], dt, tag="ps", name=_nm("ps"))

        def wload(pool, name, fi, fo, dt=F16):
            kk = fi // 128
            sb = pool.tile([128, kk * fo], dt, tag=name, name=name)
            nc.sync.dma_start(
                sb[:].rearrange("p (k f) -> p k f", k=kk, f=fo),
                w_d[name][:, :].rearrange("(k p) f -> p k f", p=128),
            )
            return sb

        # ---------- persistent small tensors ----------
        identr = persist.tile([128, 128], F16)
        rows2 = persist.tile([65, ROWS_W], F16)
        bcol = persist.tile([128, BC_N], F32)
        degc = persist.tile([128, T], F32)
        S_nm = persist.tile([128, T * K], F16)
        lgs_nm = persist.tile([128, T * K], F16)
        out_fm = persist.tile([128, 12 * G], F16)  # readout maxes, col=ch*G+g
        nmax = persist.tile([128, T], F32)
        sumx = persist.tile([128, T], F32)
        y_sb = persist.tile([128, 2 * G], F16)
        z_sb = persist.tile([10, G], F32)

        def ones_at(p, n):
            return rows2[p : p + 1, 0:n]

        def rrow(ro, n):
            p, off = ro
            return rows2[p : p + 1, off : off + n]

        # ---------- pools (opened in LIFO close order; fnmp closes first) ----------
        hres = ex.enter_context(tc.tile_pool(name="hres", bufs=1))
        xfm_p = ex.enter_context(tc.tile_pool(name="xfm", bufs=2))
        xnm_p = ex.enter_context(tc.tile_pool(name="xnm", bufs=2))
        agg_p = ex.enter_context(tc.tile_pool(name="aggfm", bufs=2))
        mid_p = ex.enter_context(tc.tile_pool(name="midp", bufs=1))
        fnm_p = ExitStack()
        fnmpool = fnm_p.enter_context(tc.tile_pool(name="fnmp", bufs=1))

        # ---------- input DMAs (issue order == priority order) ----------
        AT = persist.tile([128, T * NPG], F16, tag="AT", name="AT")
        AT8 = persist.tile([128, T * NPG], F8, tag="AT8", name="AT8")
        featnm = fnmpool.tile([128, T * IN], F16, tag="featnm")
        featT = fnmpool.tile([128, NLOC], F16, tag="featT")
        # cat8 = fp8 cat(featT, agg_feat) for the DoubleRow a1 linear
        cat8 = fnmpool.tile([128, 2 * NLOC], F8, tag="cat8", name="cat8")
        for g in range(G):
            nc.sync.dma_start(featnm[:, g * 4 * IN : (g + 1) * 4 * IN],
                              featnm_d[:, g * 4 * IN : (g + 1) * 4 * IN])
            nc.sync.dma_start(AT[:, g * 4 * NPG : (g + 1) * 4 * NPG],
                              at_d[:, g * 4 * NPG : (g + 1) * 4 * NPG])
        nc.sync.dma_start(featT[:], featT_d[:])
        W1 = wload(persist, "W1", 256, 256)
        aW1 = wload(persist, "aW1", 256, 256, F8)
        aW1v = aW1[:].rearrange("p (k f) -> p k f", k=2, f=256)
        nc.sync.dma_start(cat8[:, 0:NLOC], featT8_d[:])
        nc.sync.dma_start(identr[:], identr_d[:])
        nc.sync.dma_start(rows2[:], rows_d[:])
        nc.sync.dma_start(bcol[:], bcol_d[:])
        nc.sync.dma_start(degc[:], degc_d[:])
        nc.sync.dma_start(AT8[:], at8_d[:])
        W2 = wload(persist, "W2", 512, 256)
        aW2 = wload(persist, "aW2", 512, 256, F8)
        aW2v = aW2[:].rearrange("p (k f) -> p k f", k=4, f=256)
        W3 = wload(persist, "W3", 512, 256)
        aW3 = persist.tile([128, 4 * 2048], F8, tag="aW3", name="aW3")
        aW3v = aW3[:].rearrange("p (k f) -> p k f", k=4, f=2048)
        for q in range(4):
            nc.sync.dma_start(
                aW3v[:, :, q * 512 : (q + 1) * 512],
                w_d["aW3"][:, q * 512 : (q + 1) * 512].rearrange(
                    "(k p) f -> p k f", p=128))
        pWa = wload(persist, "pWa", 512, 256, F8)
        pWav = pWa[:].rearrange("p (k f) -> p k f", k=4, f=256)
        pW3 = wload(persist, "pW3", 2048, 256, F8)
        pW3v = pW3[:].rearrange("p (k f) -> p k f", k=16, f=256)
        qW1 = wload(persist, "qW1", 1536, 256)
        qW2 = wload(persist, "qW2", 512, 256)
        qW3 = wload(persist, "qW3", 512, 256)
        mW1 = wload(persist, "mW1", 1536, 256)
        mW2 = wload(persist, "mW2", 256, 10)

        # ---------- activation tiles ----------
        h1n = hres.tile([128, T * HID], F16, tag="h1n", name="h1n")
        h2n = hres.tile([128, T * HID], F16, tag="h2n", name="h2n")
        h3n = hres.tile([128, T * HID], F16, tag="h3n", name="h3n")
        a1f = hres.tile([128, 2 * NLOC], F16, tag="a1f", name="a1f")
        a2f = hres.tile([128, 2 * NLOC], F16, tag="a2f", name="a2f")
        a1f8 = hres.tile([128, 2 * NLOC], F8, tag="a1f8", name="a1f8")
        a2f8 = hres.tile([128, 2 * NLOC], F8, tag="a2f8", name="a2f8")
        agga18 = hres.tile([128, 2 * NLOC], F8, tag="agga18", name="agga18")
        agga28 = hres.tile([128, 2 * NLOC], F8, tag="agga28", name="agga28")

        # ---------- PSUM evacuation, round-robin across Act/DVE ----------
        rr = [0]

        def evac(dst, src, bias=None, relu=False, w=(1, 1)):
            """dst = act(src + bias); engines weighted (Act, DVE)."""
            sel = rr[0] % (w[0] + w[1])
            rr[0] += 1
            if sel < w[0]:
                nc.scalar.activation(dst, src, AF.Relu if relu else AF.Identity,
                                     bias=bias if bias is not None else 0.0)
            else:
                if bias is None and not relu:
                    nc.vector.tensor_copy(dst, src)
                elif relu:
                    nc.vector.tensor_scalar(dst, src,
                                            bias if bias is not None else 0.0,
                                            0.0, op0=ALU.add, op1=ALU.max)
                else:
                    nc.vector.tensor_scalar(dst, src, bias, None, op0=ALU.add)

        # ---------- emit helpers ----------
        def emit_agg(x_nm, D, out_t):
            """out_t[d, n] (feature-major) = sum_s x_nm[s, d] * AT[s, n]."""
            for g in range(G):
                for ch in range(D // 128):
                    ps = ps_big()
                    for st in range(4):
                        t = g * 4 + st
                        nc.tensor.matmul(
                            ps[:],
                            lhsT=x_nm[:, t * D + ch * 128 : t * D + ch * 128 + 128],
                            rhs=AT[:, t * NPG : (t + 1) * NPG],
                            start=(st == 0), stop=(st == 3))
                    evac(out_t[:, ch * NLOC + g * NPG : ch * NLOC + (g + 1) * NPG],
                         ps[:])

        def emit_agg8(x_nm8, out_t8):
            """fp8 DoubleRow aggregation (D=256): st tiles paired."""
            xv = x_nm8[:].rearrange("p (t f) -> p t f", t=T, f=HID)
            atv = AT8[:].rearrange("p (t n) -> p t n", t=T, n=NPG)
            for g in range(G):
                for ch in range(2):
                    ps = ps_big()
                    for sp in range(2):
                        t = g * 4 + 2 * sp
                        nc.tensor.matmul(
                            ps[:],
                            lhsT=xv[:, t : t + 2, ch * 128 : ch * 128 + 128],
                            rhs=atv[:, t : t + 2, :],
                            start=(sp == 0), stop=(sp == 1), perf_mode=DR)
                    evac(out_t8[:, ch * NLOC + g * NPG : ch * NLOC + (g + 1) * NPG],
                         ps[:])

        def emit_lin_fm(x_fm, a_fm, Din, Dout, Wsb, bccol, relu, out_t,
                        out2_t=None):
            nk = Din // 128
            for co in range(Dout // 128):
                for nb in range(4):
                    ps = ps_big()
                    ki = 0
                    for src in (x_fm, a_fm):
                        for ci in range(nk):
                            nc.tensor.matmul(
                                ps[:],
                                lhsT=Wsb[:, ki * Dout + co * 128 : ki * Dout + co * 128 + 128],
                                rhs=src[:, ci * NLOC + nb * 512 : ci * NLOC + (nb + 1) * 512],
                                start=(ki == 0), stop=(ki == 2 * nk - 1))
                            ki += 1
                    sl = slice(co * NLOC + nb * 512, co * NLOC + (nb + 1) * 512)
                    evac(out_t[:, sl], ps[:],
                         bias=bcol[:, bccol + co : bccol + co + 1], relu=relu)
                    if out2_t is not None:
                        evac(out2_t[:, sl], ps[:],
                             bias=bcol[:, bccol + co : bccol + co + 1], relu=relu)

        def emit_lin8(srcs, Wv, bccol, out_t):
            """fp8 DoubleRow linear: srcs = list of [128, 2*NLOC] fp8 tiles
            (each = one 256-channel pair); relu + bias via evac."""
            for co in range(2):
                for nb in range(4):
                    ps = ps_big()
                    for si, s8 in enumerate(srcs):
                        sv = s8[:].rearrange("p (c n) -> p c n", c=2, n=NLOC)
                        nc.tensor.matmul(
                            ps[:],
                            lhsT=Wv[:, 2 * si : 2 * si + 2,
                                    co * 128 : co * 128 + 128],
                            rhs=sv[:, :, nb * 512 : (nb + 1) * 512],
                            start=(si == 0), stop=(si == len(srcs) - 1),
                            perf_mode=DR)
                    sl = slice(co * NLOC + nb * 512, co * NLOC + (nb + 1) * 512)
                    evac(out_t[:, sl], ps[:],
                         bias=bcol[:, bccol + co : bccol + co + 1], relu=True)

        def emit_nm_T(x_fm, out_nm):
            # node-major via PE transposes of the (already relu'd) fm tensor;
            # 4 transposed blocks share one PSUM bank -> single 512-wide evac.
            for t2 in range(0, T, 2):
                tp = ps_big(F16)
                for i, (t, ch) in enumerate(
                        ((t2, 0), (t2, 1), (t2 + 1, 0), (t2 + 1, 1))):
                    nc.tensor.matmul(
                        tp[:, i * 128 : (i + 1) * 128],
                        lhsT=x_fm[:, ch * NLOC + t * 128 : (t + 1) * 128 + ch * NLOC],
                        rhs=identr[:], is_transpose=True,
                        start=True, stop=True, skip_group_check=True)
                evac(out_nm[:, t2 * HID : (t2 + 2) * HID], tp[:])

        def emit_out1(x_fm, ch0):
            for ci in range(2):
                xv = x_fm[:, ci * NLOC : (ci + 1) * NLOC].rearrange(
                    "p (g n) -> p g n", g=G, n=NPG)
                nc.vector.tensor_reduce(
                    out_fm[:, (ch0 + ci) * G : (ch0 + ci + 1) * G],
                    xv, axis=AX.X, op=ALU.max)

        # ---------- GC stacks ----------
        aggfeat = agg_p.tile([128, NLOC], F16, tag="agg", name="aggfeat")
        emit_agg(featnm, IN, aggfeat)
        for g in range(G):  # fp8 shadow for the a1 DoubleRow linear (GpSimd)
            nc.gpsimd.tensor_copy(
                cat8[:, NLOC + g * NPG : NLOC + (g + 1) * NPG],
                aggfeat[:, g * NPG : (g + 1) * NPG])

        h1f = xfm_p.tile([128, 2 * NLOC], F16, tag="xfm", name="h1f")
        emit_lin_fm(featT, aggfeat, 128, 256, W1, BC_B1, True, h1f)
        emit_nm_T(h1f, h1n)
        emit_out1(h1f, 0)

        a1n8 = xnm_p.tile([128, T * HID], F8, tag="xnm", name="a1n8")
        emit_lin8([cat8], aW1v, BC_AB1, a1f)
        emit_nm_T(a1f, a1n8)
        for q in range(4):  # fp8 shadow of a1f (GpSimd)
            nc.gpsimd.tensor_copy(a1f8[:, q * 1024 : (q + 1) * 1024],
                                  a1f[:, q * 1024 : (q + 1) * 1024])
        fnm_p.close()

        aggh1 = agg_p.tile([128, 2 * NLOC], F16, tag="agg2", name="aggh1")
        emit_agg(h1n, HID, aggh1)

        emit_agg8(a1n8, agga18)

        h2f = xfm_p.tile([128, 2 * NLOC], F16, tag="xfm", name="h2f")
        emit_lin_fm(h1f, aggh1, 256, 256, W2, BC_B2, True, h2f)
        emit_nm_T(h2f, h2n)
        emit_out1(h2f, 2)

        a2n8 = xnm_p.tile([128, T * HID], F8, tag="xnm", name="a2n8")
        emit_lin8([a1f8, agga18], aW2v, BC_AB2, a2f)
        emit_nm_T(a2f, a2n8)
        for q in range(4):  # fp8 shadow of a2f (GpSimd)
            nc.gpsimd.tensor_copy(a2f8[:, q * 1024 : (q + 1) * 1024],
                                  a2f[:, q * 1024 : (q + 1) * 1024])

        aggh2 = agg_p.tile([128, 2 * NLOC], F16, tag="agg2", name="aggh2")
        emit_agg(h2n, HID, aggh2)

        emit_agg8(a2n8, agga28)

        # h3: fm + readout + node-major (resident, no spill)
        h3f = xfm_p.tile([128, 2 * NLOC], F16, tag="xfm", name="h3f")
        emit_lin_fm(h2f, aggh2, 256, 256, W3, BC_B3, False, h3f)
        emit_nm_T(h3f, h3n)
        emit_out1(h3f, 4)

        # ---------- a3 + logits (streamed per graph, fp8 DoubleRow) ----------
        a2f8v = a2f8[:].rearrange("p (c n) -> p c n", c=2, n=NLOC)
        agga2v = agga28[:].rearrange("p (c n) -> p c n", c=2, n=NLOC)
        a1f8v = a1f8[:].rearrange("p (c n) -> p c n", c=2, n=NLOC)
        for g in range(G):
            lps = lg_p.tile([64, 512], F32, tag="lg", name=_nm("lg"))
            gn = slice(g * NPG, (g + 1) * NPG)
            for cop in range(8):  # a3 = relu(cat(a2, agg_a2) @ aW3 + ab3)
                ab2 = mid_p.tile([128, 2 * 512], F8, tag="a3buf",
                                 name=_nm("a3b"), bufs=3)
                psp = a3_p.tile([128, 1024], F32, tag="a3ps", name=_nm("a3p"))
                for sub in range(2):
                    co = 2 * cop + sub
                    for pair, srcv in enumerate((a2f8v, agga2v)):
                        nc.tensor.matmul(
                            psp[:, sub * 512 : (sub + 1) * 512],
                            lhsT=aW3v[:, 2 * pair : 2 * pair + 2,
                                      co * 128 : co * 128 + 128],
                            rhs=srcv[:, :, gn],
                            start=(pair == 0), stop=(pair == 1),
                            perf_mode=DR, skip_group_check=True)
                if ab3_zero:
                    evac(ab2[:], psp[:], relu=True)
                else:
                    for sub in range(2):
                        co = 2 * cop + sub
                        evac(ab2[:, sub * 512 : (sub + 1) * 512],
                             psp[:, sub * 512 : (sub + 1) * 512],
                             bias=bcol[:, BC_AB3 + co : BC_AB3 + co + 1],
                             relu=True)
                ab2v = ab2[:].rearrange("p (c n) -> p c n", c=2, n=512)
                nc.tensor.matmul(
                    lps[:],
                    lhsT=pW3v[:, 2 * cop : 2 * cop + 2, g * K : g * K + K],
                    rhs=ab2v[:], start=(cop == 0), stop=False,
                    perf_mode=DR, skip_group_check=True)
            for bi, srcv in enumerate((a1f8v, a2f8v)):  # a1/a2 blocks of pW
                nc.tensor.matmul(
                    lps[:],
                    lhsT=pWav[:, 2 * bi : 2 * bi + 2, g * K : g * K + K],
                    rhs=srcv[:, :, gn], start=False, stop=False,
                    perf_mode=DR, skip_group_check=True)
            nc.tensor.matmul(lps[:],
                             lhsT=rrow(R_PB, 256)[:, g * K : (g + 1) * K],
                             rhs=ones_at(32, 512), start=False, stop=True,
                             skip_group_check=True)
            lgf = mid_p.tile([64, 512], F16, tag="lgf", name=_nm("lgf"), bufs=1)
            evac(lgf[:], lps[:])
            for j in range(4):  # transpose to node-major
                t = g * 4 + j
                tps = ps_big(F16)
                nc.tensor.matmul(tps[:, 0:64], lhsT=lgf[0:64, j * 128 : (j + 1) * 128],
                                 rhs=identr[0:64, 0:64], is_transpose=True,
                                 start=True, stop=True)
                nc.vector.tensor_copy(lgs_nm[:, t * K : (t + 1) * K], tps[:, 0:64])

        # masked softmax == per-graph softmax over K columns
        nc.vector.tensor_reduce(
            nmax[:], lgs_nm[:].rearrange("p (t k) -> p t k", t=T, k=K),
            axis=AX.X, op=ALU.max, negate=True)
        for t in range(T):
            nc.scalar.activation(S_nm[:, t * K : (t + 1) * K],
                                 lgs_nm[:, t * K : (t + 1) * K], AF.Exp,
                                 bias=nmax[:, t : t + 1],
                                 accum_out=sumx[:, t : t + 1])
        nc.vector.reciprocal(sumx[:], sumx[:])
        for t in range(T):
            nc.vector.tensor_scalar(S_nm[:, t * K : (t + 1) * K],
                                    S_nm[:, t * K : (t + 1) * K],
                                    sumx[:, t : t + 1], None, op0=ALU.mult)

        # ---------- late pool: pooled stage ----------
        late = ex.enter_context(tc.tile_pool(name="late", bufs=1))
        Xr = [h1n, h2n, h3n]
        AS_nm = late.tile([128, T * K], F16, tag="AS", name="AS_nm")
        rs_n = late.tile([128, T], F16, tag="rsn", name="rs_n")

        # AS = A @ S: scaled-AT product un-scaled by clamped deg (exact)
        for t in range(T):
            g, j = t // 4, t % 4
            ps = ps_big()
            for st in range(4):
                nc.tensor.matmul(
                    ps[:, 0:K],
                    lhsT=AT[:, (g * 4 + st) * NPG + j * 128 : (g * 4 + st) * NPG + (j + 1) * 128],
                    rhs=S_nm[:, (g * 4 + st) * K : (g * 4 + st + 1) * K],
                    start=(st == 0), stop=(st == 3))
            nc.vector.tensor_scalar(AS_nm[:, t * K : (t + 1) * K], ps[:, 0:K],
                                    degc[:, t : t + 1], None, op0=ALU.mult)
        # rs_n[n] = sum_l AS[n, l] (for adj row sums)
        nc.vector.tensor_reduce(
            rs_n[:], AS_nm[:].rearrange("p (t k) -> p t k", t=T, k=K),
            axis=AX.X, op=ALU.add)

        # ---------- h_pool = S^T X, pair-stacked [128 = 2 graphs, .] ----------
        # gs=1 matmuls land directly on PSUM partitions 64.. (tile_position).
        hp_nm = late.tile([128, 2 * 768], F16, tag="hpn", name="hp_nm")
        hp_fm = late.tile([128, 6 * 256], F16, tag="hpf", name="hp_fm")
        for h in range(2):
            for L in range(3):
                ps = ps_big()
                for gs in range(2):
                    g = h * 2 + gs
                    for j in range(4):
                        t = g * 4 + j
                        nc.tensor.matmul(
                            ps[gs * 64 : gs * 64 + 64, 0:256],
                            lhsT=S_nm[:, t * K : (t + 1) * K],
                            rhs=Xr[L][:, t * HID : (t + 1) * HID],
                            start=(j == 0), stop=(j == 3),
                            skip_group_check=True)
                evac(hp_nm[:, h * 768 + L * 256 : h * 768 + (L + 1) * 256],
                     ps[:, 0:256])
            for ch in range(6):  # hp_fm via transposes of the pair tile
                tp = ps_big(F16)
                nc.tensor.matmul(
                    tp[:, 0:128],
                    lhsT=hp_nm[:, h * 768 + ch * 128 : h * 768 + (ch + 1) * 128],
                    rhs=identr[:], is_transpose=True, start=True, stop=True)
                nc.vector.tensor_copy(
                    hp_fm[:, ch * 256 + h * 128 : ch * 256 + (h + 1) * 128],
                    tp[:, 0:128])

        # ---------- adjT = (AS)^T S directly (block-diag, pair-stacked) ----------
        # row sums of adj via rsum_row[1, K] = sum_n rs_n[n] S[n, k];
        # normalization applied as a column scale through a ones-outer-product.
        adjT = late.tile([128, 2 * 128], F16, tag="adjT", name="adjT")
        rrec = late.tile([1, 256], F16, tag="rrec", name="rrec")
        nc.vector.memset(adjT[:], 0.0)
        for h in range(2):
            pt = ps_big()
            pr = ps_big()
            for gs in range(2):
                g = h * 2 + gs
                for j in range(4):
                    t = g * 4 + j
                    nc.tensor.matmul(
                        pt[gs * 64 : gs * 64 + 64, gs * 64 : gs * 64 + 64],
                        lhsT=AS_nm[:, t * K : (t + 1) * K],
                        rhs=S_nm[:, t * K : (t + 1) * K],
                        start=(j == 0), stop=(j == 3), skip_group_check=True)
                    nc.tensor.matmul(
                        pr[0:1, gs * K : (gs + 1) * K],
                        lhsT=rs_n[:, t : t + 1],
                        rhs=S_nm[:, t * K : (t + 1) * K],
                        start=(j == 0), stop=(j == 3), skip_group_check=True)
            for gs in range(2):
                nc.vector.tensor_copy(
                    adjT[gs * 64 : gs * 64 + 64,
                         h * 128 + gs * 64 : h * 128 + gs * 64 + 64],
                    pt[gs * 64 : gs * 64 + 64, gs * 64 : gs * 64 + 64])
            nc.vector.tensor_scalar(rrec[:, h * 128 : (h + 1) * 128],
                                    pr[0:1, 0:128], 1e-9, None, op0=ALU.add)
            nc.vector.reciprocal(rrec[:, h * 128 : (h + 1) * 128],
                                 rrec[:, h * 128 : (h + 1) * 128])
        bcst = late.tile([128, 256], F16, tag="bcst", name="bcst")
        for h in range(2):
            pb = ps_big()
            nc.tensor.matmul(pb[:, 0:128], lhsT=ones_at(0, 128),
                             rhs=rrec[:, h * 128 : (h + 1) * 128],
                             start=True, stop=True)
            nc.scalar.copy(bcst[:, h * 128 : (h + 1) * 128], pb[:, 0:128])
        nc.vector.tensor_tensor(adjT[:], in0=adjT[:], in1=bcst[:], op=ALU.mult)

        # ---------- pooled sage stack (pair-batched) ----------
        hn1_fm = late.tile([128, 6 * 256], F16, tag="hn1", name="hn1_fm")
        p1_nm = late.tile([128, 2 * 256], F16, tag="p1n", name="p1_nm")
        p1_fm = late.tile([128, 2 * 256], F16, tag="p1f", name="p1_fm")
        hn2_fm = late.tile([128, 2 * 256], F16, tag="hn2", name="hn2_fm")
        p2_nm = late.tile([128, 2 * 256], F16, tag="p2n", name="p2_nm")
        p2_fm = late.tile([128, 2 * 256], F16, tag="p2f", name="p2_fm")
        hn3_fm = late.tile([128, 2 * 256], F16, tag="hn3", name="hn3_fm")
        p3_fm = late.tile([128, 2 * 256], F16, tag="p3f", name="p3_fm")

        def pool_hn(x_nm, xw, out_t):
            # out[d, u-pair] = sum_{v-pair} x_nm[v, d] * adjT_bd[v, u]
            for h in range(2):
                for ch in range(xw // 128):
                    tp = ps_big()
                    nc.tensor.matmul(
                        tp[:, 0:128],
                        lhsT=x_nm[:, h * xw + ch * 128 : h * xw + (ch + 1) * 128],
                        rhs=adjT[:, h * 128 : (h + 1) * 128],
                        start=True, stop=True)
                    evac(out_t[:, ch * 256 + h * 128 : ch * 256 + (h + 1) * 128],
                         tp[:, 0:128])

        def pool_lin(xf, hf, Din, Wsb, bccol, rbias, relu, outf, outn):
            nch = Din // 256
            for co in range(2):
                ps = ps_big()
                ki = 0
                for src in (xf, hf):
                    for ch in range(nch):
                        nc.tensor.matmul(
                            ps[:, 0:256],
                            lhsT=Wsb[:, ki * 256 + co * 128 : ki * 256 + co * 128 + 128],
                            rhs=src[:, ch * 256 : (ch + 1) * 256],
                            start=(ki == 0), stop=(ki == 2 * nch - 1))
                        ki += 1
                evac(outf[:, co * 256 : (co + 1) * 256], ps[:, 0:256],
                     bias=bcol[:, bccol + co : bccol + co + 1], relu=relu)
            if outn is not None:
                for h in range(2):
                    ps = ps_big()
                    ki = 0
                    for src in (xf, hf):
                        for ch in range(nch):
                            nc.tensor.matmul(
                                ps[:, 0:256],
                                lhsT=src[:, ch * 256 + h * 128 : ch * 256 + (h + 1) * 128],
                                rhs=Wsb[:, ki * 256 : (ki + 1) * 256],
                                start=(ki == 0), stop=False)
                            ki += 1
                    nc.tensor.matmul(ps[:, 0:256], lhsT=ones_at(rbias[0], 128),
                                     rhs=rrow(rbias, 256),
                                     start=False, stop=True)
                    nc.vector.tensor_scalar(outn[:, h * 256 : (h + 1) * 256],
                                            ps[:, 0:256],
                                            0.0, None, op0=ALU.max)

        pool_hn(hp_nm, 768, hn1_fm)
        pool_lin(hp_fm, hn1_fm, 1536, qW1, BC_QB1, R_QB1, True, p1_fm, p1_nm)
        pool_hn(p1_nm, 256, hn2_fm)
        pool_lin(p1_fm, hn2_fm, 512, qW2, BC_QB2, R_QB2, True, p2_fm, p2_nm)
        pool_hn(p2_nm, 256, hn3_fm)
        pool_lin(p2_fm, hn3_fm, 512, qW3, BC_QB3, R_QB3, False, p3_fm, None)
        for L, pf in enumerate((p1_fm, p2_fm, p3_fm)):
            for co in range(2):
                xv = pf[:, co * 256 : (co + 1) * 256].rearrange(
                    "p (g k) -> p g k", g=G, k=K)
                nc.vector.tensor_reduce(
                    out_fm[:, (6 + L * 2 + co) * G : (6 + L * 2 + co + 1) * G],
                    xv, axis=AX.X, op=ALU.max)

        # ---------- final MLP ----------
        for co in range(2):
            ps = ps_big()
            for k in range(12):
                nc.tensor.matmul(
                    ps[:, 0:G],
                    lhsT=mW1[:, k * 256 + co * 128 : k * 256 + co * 128 + 128],
                    rhs=out_fm[:, k * G : (k + 1) * G],
                    start=(k == 0), stop=(k == 11))
            nc.scalar.activation(y_sb[:, co * G : (co + 1) * G], ps[:, 0:G],
                                 AF.Identity,
                                 bias=bcol[:, BC_MB1 + co : BC_MB1 + co + 1])
        zps = ps_big()
        for ci in range(2):
            nc.tensor.matmul(zps[0:10, 0:G], lhsT=mW2[:, ci * 10 : (ci + 1) * 10],
                             rhs=y_sb[:, ci * G : (ci + 1) * G],
                             start=(ci == 0), stop=(ci == 1))
        nc.scalar.activation(z_sb[:], zps[0:10, 0:G], AF.Identity,
                             bias=bcol[0:10, BC_MB2 : BC_MB2 + 1])
        nc.sync.dma_start(yp_d[:], z_sb[:])

    nc.compile()
    return nc


# ---------------------------------------------------------------------------
# host side
# ---------------------------------------------------------------------------

def _pack_bcol(b):
    bc = np.zeros((128, BC_N), np.float32)
    for off, k in ((BC_B1, "b1"), (BC_B2, "b2"), (BC_B3, "b3"), (BC_AB1, "ab1"),
                   (BC_AB2, "ab2"), (BC_AB3, "ab3"), (BC_QB1, "qb1"),
                   (BC_QB2, "qb2"), (BC_QB3, "qb3"), (BC_MB1, "mb1")):
        v = np.asarray(b[k], np.float32)
        bc[:, off : off + v.size // 128] = v.reshape(-1, 128).T
    mb2 = np.asarray(b["mb2"], np.float32)
    bc[: mb2.size, BC_MB2] = mb2
    return bc


def _pack_rows(b, pb_lc):
    r = np.zeros((65, ROWS_W), np.float32)
    for p in (0, 32, 64):
        r[p, 0:512] = 1.0
    for (p, off), k in ((R_QB1, "qb1"), (R_QB2, "qb2"), (R_QB3, "qb3")):
        r[p, off : off + 256] = b[k]
    p, off = R_PB
    r[p, off : off + 256] = pb_lc
    return r.astype(np.float16)


def _at_dense(edge_src, edge_dst, core):
    """Dense scaled A^T tiles [128, T*NPG] fp16 plus clamped-deg cols."""
    lo, hi = core * NLOC, (core + 1) * NLOC
    m = (edge_dst >= lo) & (edge_dst < hi)
    src = edge_src[m].astype(np.int64)
    dst = edge_dst[m].astype(np.int64)
    gg = dst // NPG
    if not np.array_equal(src // NPG, gg):
        raise ValueError("cross-graph edges break graph-parallel sharding")
    gl = gg - core * G
    sl = src - gg * NPG
    dl = dst - gg * NPG
    t = gl * 4 + sl // 128
    p = sl % 128
    flat = (p * T + t) * NPG + dl
    cnt = np.bincount(flat, minlength=128 * T * NPG).astype(np.float64)
    at = cnt.reshape(128, T * NPG)
    # deg per local node (node-major: node = tt*128 + pp)
    nl = gl * NPG + dl
    deg = np.bincount(nl, minlength=NLOC).astype(np.float64)
    degc = np.maximum(deg, 1.0)
    # scale each AT column (dst d of graph g == local node g*NPG+d)
    colnode = (np.arange(T * NPG) // (4 * NPG)) * NPG + np.arange(T * NPG) % NPG
    at = at / degc[colnode][None, :]
    degc_nm = degc.reshape(T, 128).T.astype(np.float32)
    return at.astype(np.float16), np.ascontiguousarray(degc_nm)


_CACHE = {}
TRACE = False


def prepare_in_maps(inputs):
    import ml_dtypes
    f16 = lambda x: np.ascontiguousarray(np.asarray(x, np.float32).astype(np.float16))
    f8 = lambda x: np.ascontiguousarray(
        np.asarray(x, np.float32).astype(ml_dtypes.float8_e4m3))
    feat = np.asarray(inputs["feat"], np.float32)
    edge_src = np.asarray(inputs["edge_src"])
    edge_dst = np.asarray(inputs["edge_dst"])
    W16 = {k: f16(inputs[k]) for k in
           ("W1", "W2", "W3", "qW1", "qW2", "qW3", "mW1", "mW2")}
    W8 = {k: f8(inputs[k]) for k in ("aW1", "aW2", "aW3")}
    pW = f16(inputs["pW"])
    b = {k: np.asarray(inputs[k], np.float32) for k in
         ("b1", "b2", "b3", "ab1", "ab2", "ab3", "pb", "qb1", "qb2", "qb3",
          "mb1", "mb2")}
    identr = np.eye(128, dtype=np.float16)
    bcol = _pack_bcol(b)

    in_maps = []
    for c in range(NCORES):
        fs = feat[c * NLOC : (c + 1) * NLOC]
        feat_nm = np.ascontiguousarray(
            fs.reshape(T, 128, IN).transpose(1, 0, 2).reshape(128, T * IN))
        featT = np.ascontiguousarray(fs.T)
        at, degc = _at_dense(edge_src, edge_dst, c)
        pW_lc = np.ascontiguousarray(pW[:, c * G * K : (c + 1) * G * K])
        pb_lc = np.ascontiguousarray(b["pb"][c * G * K : (c + 1) * G * K])
        in_maps.append({
            "featT": f16(featT), "featT8": f8(featT), "feat_nm": f16(feat_nm),
            "at_dense": at, "at8": f8(at.astype(np.float32)), "degc": degc,
            "bcol": bcol, "rows2": _pack_rows(b, pb_lc),
            "identr": identr,
            "W1": W16["W1"], "W2": W16["W2"], "W3": W16["W3"],
            "aW1": W8["aW1"], "aW2": W8["aW2"], "aW3": W8["aW3"],
            "pWa": f8(pW_lc[:512]),
            "pW3": f8(pW_lc[512:]),
            "qW1": W16["qW1"], "qW2": W16["qW2"], "qW3": W16["qW3"],
            "mW1": W16["mW1"], "mW2": W16["mW2"],
        })
    return in_maps


def kernel(**inputs):
    ab3_zero = not np.any(np.asarray(inputs["ab3"], np.float32))
    if _CACHE.get("ab3_zero") != ab3_zero or "nc" not in _CACHE:
        _CACHE["nc"] = build_module(ab3_zero)
        _CACHE["ab3_zero"] = ab3_zero
    nc = _CACHE["nc"]
    in_maps = prepare_in_maps(inputs)
    res = run_bass_kernel_spmd(nc, in_maps, core_ids=list(range(NCORES)),
                               trace=TRACE)
    _CACHE["last_res"] = res
    out = np.zeros((B, 10), np.float32)
    for c in range(NCORES):
        out[c * G : (c + 1) * G, :] = np.asarray(res.results[c]["yp"]).T
    return out
